# revision 22
# baseline (speedup 1.0000x reference)
"""Multi-head causal attention (B=2, S=2048, H=1024, 16 heads) on 8 TRN2
NeuronCores.

Sharding: core c in 0..7 handles batch b = c // 4 and head group g = c % 4
(heads 4g..4g+3).  Each core computes Q/K/V projections for its 4 heads,
causal attention, and the partial output projection through its column slice
of Wo.  The 4 cores of a batch ReduceScatter(add) their [2048, 1024] partials
so core i of the group ends up with rows 512*i..512*i+512 fully reduced; the
host concatenates the chunks.

Device dataflow (per core, all matmuls in float32r):
  - activations pre-transposed on host to [1024, 2048] (feature-major) since
    the PE contracts over the partition dim
  - QT/KT [256, 2048] via weight-stationary matmuls, bias fused in the ACT
    PSUM->SBUF evacuation; V [2048, 256] natural with a fused ones column per
    head (rowsums fall out of the attention-value matmul)
  - scores computed transposed (scoresT[k, q]) so the exp'd tiles feed the
    attention-value matmul directly as the stationary operand, no transposes
  - causal handled by skipping fully-masked 128x512 blocks and multiplying
    the 4 diagonal-block patterns with precomputed 0/1 masks
  - softmax normalization: rowsum row (partition 64 of the attnV PSUM tile)
    -> GpSimd partition_broadcast -> DVE reciprocal approx -> multiply during
    PSUM evacuation; 1/sqrt(64) folded into the exp activation scale
  - out projection contracts per-head (K=64) over host-split Wo slices; bias
    bo added after the ReduceScatter on each core's chunk
"""

import sys

for _p in ("/opt/trn_rl_repo", "/root/.axon_site/_ro/trn_rl_repo"):
    if _p not in sys.path:
        sys.path.insert(0, _p)

import numpy as np

import concourse.bass as bass
import concourse.tile as tile
from concourse import bacc
import concourse.mybir as mybir

B = 2
S = 2048
HID = 1024
HEADS_PER_CORE = 4
DH = 64  # head dim
HG = HEADS_PER_CORE * DH  # 256: hidden slice per core
N_CORES = 8
GROUP = 4  # cores per batch (reduction group)

F32 = mybir.dt.float32
F32R = mybir.dt.float32r
AF = mybir.ActivationFunctionType
ALU = mybir.AluOpType

KT = 128  # contraction tile (partitions)
QS = 512  # q strip width
NKT = S // KT  # 16 k-tiles
NQS = S // QS  # 4 q strips
NST = S // KT  # 16 s tiles


def build_nc():
    nc = bacc.Bacc(
        "TRN2", target_bir_lowering=False, debug=False, num_devices=N_CORES
    )

    # per-core inputs (already sharded/transposed by the host)
    xq = nc.dram_tensor("xq", [HID, S], F32, kind="ExternalInput").ap()
    xk = nc.dram_tensor("xk", [HID, S], F32, kind="ExternalInput").ap()
    xv = nc.dram_tensor("xv", [HID, S], F32, kind="ExternalInput").ap()
    wq = nc.dram_tensor("wq", [HID, HG], F32, kind="ExternalInput").ap()
    wk = nc.dram_tensor("wk", [HID, HG], F32, kind="ExternalInput").ap()
    wv = nc.dram_tensor("wv", [HID, HG], F32, kind="ExternalInput").ap()
    w2 = nc.dram_tensor("w2", [HEADS_PER_CORE, DH, HID], F32, kind="ExternalInput").ap()
    bqk = nc.dram_tensor("bqk", [2, 2, 128, 1], F32, kind="ExternalInput").ap()
    bvb = nc.dram_tensor("bvb", [128, HG], F32, kind="ExternalInput").ap()
    bob = nc.dram_tensor("bob", [128, HID], F32, kind="ExternalInput").ap()
    msk = nc.dram_tensor("msk", [128, 4, QS], F32, kind="ExternalInput").ap()

    out_chunk = nc.dram_tensor(
        "out_chunk", [S // GROUP, HID], F32, kind="ExternalOutput"
    ).ap()

    out_part = nc.dram_tensor("out_part", [S, HID], F32)
    rs_out = nc.dram_tensor("rs_out", [S // GROUP, HID], F32)

    groups = [[0, 1, 2, 3], [4, 5, 6, 7]]

    with tile.TileContext(nc) as tc:
        with (
            tc.tile_pool(name="wpool", bufs=1) as wpool,
            tc.tile_pool(name="qkv", bufs=1) as qkv,
        ):
            # ---- constants / weights ----
            wq_sb = []
            wk_sb = []
            wv_sb = []
            for k in range(8):
                t = wpool.tile([128, HG], F32R, tag=f"wq{k}")
                nc.sync.dma_start(t[:], wq[128 * k : 128 * k + 128].bitcast(F32R))
                wq_sb.append(t)
                t = wpool.tile([128, HG], F32R, tag=f"wk{k}")
                nc.sync.dma_start(t[:], wk[128 * k : 128 * k + 128].bitcast(F32R))
                wk_sb.append(t)
                t = wpool.tile([128, HG], F32R, tag=f"wv{k}")
                nc.sync.dma_start(t[:], wv[128 * k : 128 * k + 128].bitcast(F32R))
                wv_sb.append(t)
            w2_sb = []
            for h in range(HEADS_PER_CORE):
                t = wpool.tile([DH, HID], F32R, tag=f"w2{h}")
                nc.sync.dma_start(t[:], w2[h].bitcast(F32R))
                w2_sb.append(t)
            bq_sb = []
            bk_sb = []
            for m in range(2):
                t = wpool.tile([128, 1], F32, tag=f"bq{m}")
                nc.sync.dma_start(t[:], bqk[0, m])
                bq_sb.append(t)
                t = wpool.tile([128, 1], F32, tag=f"bk{m}")
                nc.sync.dma_start(t[:], bqk[1, m])
                bk_sb.append(t)
            bv_sb = wpool.tile([128, HG], F32, tag="bvb")
            nc.sync.dma_start(bv_sb[:], bvb[:])
            bo_sb = wpool.tile([128, HID], F32, tag="bob")
            nc.sync.dma_start(bo_sb[:], bob[:])
            mask_sb = wpool.tile([128, 4, QS], F32R, tag="msk")
            nc.sync.dma_start(mask_sb[:], msk.bitcast(F32R))

            # ---- persistent activations ----
            # QT/KT: [dh', s] with heads 2t, 2t+1 in partition halves of tile t
            qt_sb = [qkv.tile([128, S], F32R, tag=f"qt{m}", name=f"qt{m}") for m in range(2)]
            kt_sb = [qkv.tile([128, S], F32R, tag=f"kt{m}", name=f"kt{m}") for m in range(2)]
            # V natural [s, (head, dh+1)] with a ones column per head
            v_sb = [qkv.tile([128, HEADS_PER_CORE, DH + 1], F32R, tag=f"v{st}", name=f"v{st}")
                    for st in range(NST)]
            # normalized attention outputs OT, per (head, strip): [dh, q]
            ot_sb = [[qkv.tile([DH, QS], F32R, tag=f"ot{h}_{s4}", name=f"ot{h}_{s4}")
                      for s4 in range(NQS)] for h in range(HEADS_PER_CORE)]

            # ---- phase P: projections ----
            with tc.tile_pool(name="xs", bufs=4) as xs, \
                 tc.tile_pool(name="pj", bufs=2, space="PSUM") as pj, \
                 tc.tile_pool(name="pv", bufs=4, space="PSUM") as pv:
                for t in range(NQS):
                    sl = slice(QS * t, QS * t + QS)
                    # QT / KT strips, weight-stationary; k-tiles streamed
                    for w_sb, xdram, sbuf, b_sb, xtag in (
                        (wq_sb, xq, qt_sb, bq_sb, "xq"),
                        (wk_sb, xk, kt_sb, bk_sb, "xk"),
                    ):
                        ps0 = pj.tile([128, QS], F32, tag="pj", name="ps0")
                        ps1 = pj.tile([128, QS], F32, tag="pj", name="ps1")
                        pboth = (ps0, ps1)
                        for k in range(8):
                            xt_ = xs.tile([128, QS], F32R, tag=xtag, name="xt")
                            dma_eng = nc.sync if k % 2 == 0 else nc.scalar
                            dma_eng.dma_start(
                                xt_[:],
                                xdram[128 * k : 128 * k + 128, sl].bitcast(F32R),
                            )
                            for m in range(2):
                                nc.tensor.matmul(
                                    pboth[m][:],
                                    w_sb[k][:, 128 * m : 128 * m + 128],
                                    xt_[:],
                                    start=(k == 0),
                                    stop=(k == 7),
                                )
                        for m in range(2):
                            nc.scalar.activation(
                                sbuf[m][:, sl], pboth[m][:], AF.Identity,
                                bias=b_sb[m][:],
                            )
                    # V s-tiles (x-stationary); k-tiles streamed, 4 psum groups
                    psv = [pv.tile([128, HG], F32, tag="pv", name=f"psv{u}")
                           for u in range(4)]
                    for k in range(8):
                        xt_ = xs.tile([128, QS], F32R, tag="xv", name="xt")
                        dma_eng = nc.sync if k % 2 == 0 else nc.scalar
                        dma_eng.dma_start(
                            xt_[:], xv[128 * k : 128 * k + 128, sl].bitcast(F32R)
                        )
                        for u in range(4):
                            nc.tensor.matmul(
                                psv[u][:],
                                xt_[:, 128 * u : 128 * u + 128],
                                wv_sb[k][:],
                                start=(k == 0),
                                stop=(k == 7),
                            )
                    for u in range(4):
                        st = 4 * t + u
                        for h in range(HEADS_PER_CORE):
                            nc.vector.tensor_tensor(
                                v_sb[st][:, h, 0:DH],
                                psv[u][:, DH * h : DH * h + DH],
                                bv_sb[:, DH * h : DH * h + DH],
                                ALU.add,
                            )
                        nc.scalar.activation(
                            v_sb[st][:, :, DH],
                            bo_sb[:, 0:HEADS_PER_CORE],
                            AF.Identity,
                            scale=0.0,
                            bias=1.0,
                        )

            # ---- phases A+O: attention, out-projection, chunked RS ----
            # s4-outer so each q-strip's output rows complete early and the
            # ReduceScatter chunks overlap with later strips' compute.
            with tc.tile_pool(name="attn", bufs=8) as attn_pool, \
                 tc.tile_pool(name="norm", bufs=2) as norm_pool, \
                 tc.tile_pool(name="osb", bufs=3) as osb_pool, \
                 tc.tile_pool(name="ps", bufs=4, space="PSUM") as psp, \
                 tc.tile_pool(name="po", bufs=2, space="PSUM") as pop, \
                 tc.tile_pool(name="pu", bufs=2, space="PSUM") as pup:
                for s4 in range(NQS):
                    qsl = slice(QS * s4, QS * s4 + QS)
                    nkt = 4 * s4 + 4
                    for pair in range(2):
                        # heads 2*pair and 2*pair+1 packed into PE row groups
                        pso = [
                            pop.tile([DH + 1, QS], F32, tag="po", name=f"pso{hh}")
                            for hh in range(2)
                        ]
                        ats = [None, None]
                        for j in range(nkt):
                            for hh in range(2):
                                h = 2 * pair + hh
                                hp = 64 * hh
                                pss = psp.tile([128, QS], F32, tag="ps", name="pss")
                                nc.tensor.matmul(
                                    pss[:],
                                    kt_sb[pair][hp : hp + 64, 128 * j : 128 * j + 128],
                                    qt_sb[pair][hp : hp + 64, qsl],
                                    start=True,
                                    stop=True,
                                )
                                at = attn_pool.tile([128, QS], F32R, tag="at", name="at")
                                nc.scalar.activation(
                                    at[:], pss[:], AF.Exp, scale=1.0 / 8.0
                                )
                                ats[hh] = at
                            # the two heads' mask+attnV
                            for hh in range(2):
                                h = 2 * pair + hh
                                at = ats[hh]
                                if j >= 4 * s4:
                                    i = j - 4 * s4
                                    nc.vector.tensor_tensor(
                                        at[:], at[:], mask_sb[:, i, :],
                                        ALU.mult,
                                    )
                                nc.tensor.matmul(
                                    pso[hh][:],
                                    v_sb[j][:, h, :],
                                    at[:],
                                    start=(j == 0),
                                    stop=(j == nkt - 1),
                                )
                        for hh in range(2):
                            h = 2 * pair + hh
                            rs = norm_pool.tile([1, QS], F32, tag="rs", name="rs")
                            nc.scalar.activation(rs[:], pso[hh][DH : DH + 1], AF.Copy)
                            rbc = norm_pool.tile([64, QS], F32, tag="rbc", name="rbc")
                            nc.gpsimd.partition_broadcast(rbc[:], rs[:])
                            rrec = norm_pool.tile([64, QS], F32, tag="rrec", name="rrec")
                            nc.vector.reciprocal_approx_fast(rrec[:], rbc[:])
                            nc.vector.tensor_tensor(
                                ot_sb[h][s4][:], pso[hh][0:DH], rrec[:], ALU.mult
                            )

                    # out-projection for this strip's four s-tiles
                    for u in range(4):
                        st = 4 * s4 + u
                        o = u * 128
                        for eh in range(2):
                            esl = slice(QS * eh, QS * eh + QS)
                            ps = pup.tile([128, QS], F32, tag="pu", name="psu")
                            for h in range(HEADS_PER_CORE):
                                nc.tensor.matmul(
                                    ps[:],
                                    ot_sb[h][s4][:, o : o + 128],
                                    w2_sb[h][:, esl],
                                    start=(h == 0),
                                    stop=(h == HEADS_PER_CORE - 1),
                                )
                            osb = osb_pool.tile([128, QS], F32, tag="osb", name="osb")
                            nc.scalar.activation(osb[:], ps[:], AF.Copy)
                            nc.sync.dma_start(
                                out_part[128 * st : 128 * st + 128, esl], osb[:]
                            )

                    # reduce-scatter this strip's 512 output rows; core with
                    # group rank r receives rows [512*s4 + 128*r, +128)
                    nc.gpsimd.collective_compute(
                        "ReduceScatter",
                        ALU.add,
                        replica_groups=groups,
                        ins=[out_part[QS * s4 : QS * s4 + QS].opt()],
                        outs=[rs_out[128 * s4 : 128 * s4 + 128].opt()],
                    )
                    # post-RS path entirely on GpSimd so no compute engine
                    # (PE/ACT/DVE) ever waits on the collective
                    t_in = osb_pool.tile([128, HID], F32, tag="rs_in", name="rs_in")
                    nc.gpsimd.dma_start(t_in[:], rs_out[128 * s4 : 128 * s4 + 128])
                    t_out = osb_pool.tile([128, HID], F32, tag="rs_bias", name="rs_b")
                    nc.gpsimd.tensor_tensor(t_out[:], t_in[:], bo_sb[:], ALU.add)
                    nc.gpsimd.dma_start(out_chunk[128 * s4 : 128 * s4 + 128], t_out[:])

    nc.compile()
    return nc


_NC = None
_RUNNER = None


def _get_runner():
    """Build the compiled 8-core PJRT callable once and cache it."""
    global _NC, _RUNNER
    if _RUNNER is not None:
        return _RUNNER

    import jax
    import numpy as _np
    from jax.sharding import Mesh, PartitionSpec
    from jax.experimental.shard_map import shard_map
    from concourse.bass2jax import (
        _bass_exec_p,
        install_neuronx_cc_hook,
        partition_id_tensor,
    )

    _NC = build_nc()
    nc = _NC
    install_neuronx_cc_hook()

    partition_name = nc.partition_id_tensor.name if nc.partition_id_tensor else None
    in_names = []
    out_names = []
    out_avals = []
    zero_outs = []
    for alloc in nc.m.functions[0].allocations:
        if not isinstance(alloc, mybir.MemoryLocationSet):
            continue
        name = alloc.memorylocations[0].name
        if alloc.kind == "ExternalInput":
            if name != partition_name:
                in_names.append(name)
        elif alloc.kind == "ExternalOutput":
            shape = tuple(alloc.tensor_shape)
            dtype = mybir.dt.np(alloc.dtype)
            out_names.append(name)
            out_avals.append(jax.core.ShapedArray(shape, dtype))
            zero_outs.append(_np.zeros(shape, dtype))
    n_params = len(in_names)
    n_outs = len(out_avals)
    all_in_names = list(in_names) + list(out_names)
    if partition_name is not None:
        all_in_names.append(partition_name)
    donate = tuple(range(n_params, n_params + n_outs))

    def _body(*args):
        operands = list(args)
        if partition_name is not None:
            operands.append(partition_id_tensor())
        outs = _bass_exec_p.bind(
            *operands,
            out_avals=tuple(out_avals),
            in_names=tuple(all_in_names),
            out_names=tuple(out_names),
            lowering_input_output_aliases=(),
            sim_require_finite=True,
            sim_require_nnan=True,
            nc=nc,
        )
        return tuple(outs)

    devices = jax.devices()[:N_CORES]
    mesh = Mesh(np.asarray(devices), ("core",))
    in_specs = (PartitionSpec("core"),) * (n_params + n_outs)
    out_specs = (PartitionSpec("core"),) * n_outs
    sharded = jax.jit(
        shard_map(
            _body, mesh=mesh, in_specs=in_specs, out_specs=out_specs, check_rep=False
        ),
        keep_unused=True,
    )

    def run(in_maps):
        per_core = [[_np.asarray(m[name]) for name in in_names] for m in in_maps]
        concat_in = [
            _np.concatenate([per_core[c][i] for c in range(N_CORES)], axis=0)
            for i in range(n_params)
        ]
        concat_zeros = [
            _np.zeros((N_CORES * z.shape[0], *z.shape[1:]), z.dtype)
            for z in zero_outs
        ]
        out_arrs = sharded(*concat_in, *concat_zeros)
        return [
            {
                name: _np.asarray(out_arrs[i]).reshape(
                    N_CORES, *out_avals[i].shape
                )[c]
                for i, name in enumerate(out_names)
            }
            for c in range(N_CORES)
        ]

    _RUNNER = run
    return run


def make_in_maps(query, key, value, Wq, bq, Wk, bk, Wv, bv, Wo, bo):
    query = np.asarray(query, dtype=np.float32)
    key = np.asarray(key, dtype=np.float32)
    value = np.asarray(value, dtype=np.float32)
    Wq = np.asarray(Wq, dtype=np.float32)
    bq = np.asarray(bq, dtype=np.float32)
    Wk = np.asarray(Wk, dtype=np.float32)
    bk = np.asarray(bk, dtype=np.float32)
    Wv = np.asarray(Wv, dtype=np.float32)
    bv = np.asarray(bv, dtype=np.float32)
    Wo = np.asarray(Wo, dtype=np.float32)
    bo = np.asarray(bo, dtype=np.float32)

    xqT = [np.ascontiguousarray(query[b].T) for b in range(B)]
    xkT = [np.ascontiguousarray(key[b].T) for b in range(B)]
    xvT = [np.ascontiguousarray(value[b].T) for b in range(B)]

    # diagonal-block causal masks: mask[k, i, q] = 1 if q >= k + 128*i
    k_idx = np.arange(128)[:, None, None]
    i_idx = np.arange(4)[None, :, None]
    q_idx = np.arange(QS)[None, None, :]
    masks = (q_idx >= k_idx + 128 * i_idx).astype(np.float32)

    bo_b = np.ascontiguousarray(np.broadcast_to(bo, (128, HID)))

    in_maps = []
    for c in range(N_CORES):
        b = c // GROUP
        g = c % GROUP
        hsl = slice(HG * g, HG * g + HG)
        wq_g = np.ascontiguousarray(Wq[hsl].T)  # [1024, 256]
        wk_g = np.ascontiguousarray(Wk[hsl].T)
        wv_g = np.ascontiguousarray(Wv[hsl].T)
        # w2[h] = Wo[:, g*256 + 64h : +64].T  -> [64, 1024]
        w2_g = np.ascontiguousarray(
            Wo[:, hsl].T.reshape(HEADS_PER_CORE, DH, HID)
        )
        bqk_g = np.stack(
            [bq[hsl].reshape(2, 128), bk[hsl].reshape(2, 128)]
        )  # [2, 2, 128]
        bv_b = np.ascontiguousarray(np.broadcast_to(bv[hsl], (128, HG)))
        in_maps.append(
            {
                "xq": xqT[b],
                "xk": xkT[b],
                "xv": xvT[b],
                "wq": wq_g,
                "wk": wk_g,
                "wv": wv_g,
                "w2": w2_g,
                "bqk": bqk_g,
                "bvb": bv_b,
                "bob": bo_b,
                "msk": masks,
            }
        )
    return in_maps


def assemble_output(results):
    # core with group rank r holds, for each strip j, global rows
    # 512*j + 128*r .. +128 in its out_chunk block j
    out = np.empty((B, S, HID), dtype=np.float32)
    for b in range(B):
        for r in range(GROUP):
            chunk = results[GROUP * b + r]["out_chunk"]
            for j in range(NQS):
                out[b, QS * j + 128 * r : QS * j + 128 * r + 128] = chunk[
                    128 * j : 128 * j + 128
                ]
    return out


def kernel(**inputs) -> np.ndarray:
    in_maps = make_in_maps(**inputs)
    run = _get_runner()
    results = run(in_maps)
    return assemble_output(results)


if __name__ == "__main__":
    import reference

    inputs = {k: np.asarray(v) for k, v in reference.setup_inputs().items()}
    got = kernel(**inputs)
    want = np.asarray(reference.reference(**inputs))
    err = np.linalg.norm(got - want) / np.linalg.norm(want)
    print("Relative error:", err)


# revision 24
# speedup vs baseline: 1.0883x; 1.0883x over previous
"""Multi-head causal attention (B=2, S=2048, H=1024, 16 heads) on 8 TRN2
NeuronCores.

Sharding: core c in 0..7 handles batch b = c // 4 and head group g = c % 4
(heads 4g..4g+3).  Each core computes Q/K/V projections for its 4 heads,
causal attention, and the partial output projection through its column slice
of Wo.  The 4 cores of a batch ReduceScatter(add) their [2048, 1024] partials
so core i of the group ends up with rows 512*i..512*i+512 fully reduced; the
host concatenates the chunks.

Device dataflow (per core, all matmuls in float32r):
  - activations pre-transposed on host to [1024, 2048] (feature-major) since
    the PE contracts over the partition dim
  - QT/KT [256, 2048] via weight-stationary matmuls, bias fused in the ACT
    PSUM->SBUF evacuation; V [2048, 256] natural with a fused ones column per
    head (rowsums fall out of the attention-value matmul)
  - scores computed transposed (scoresT[k, q]) so the exp'd tiles feed the
    attention-value matmul directly as the stationary operand, no transposes
  - causal handled by skipping fully-masked 128x512 blocks and multiplying
    the 4 diagonal-block patterns with precomputed 0/1 masks
  - softmax normalization: rowsum row (partition 64 of the attnV PSUM tile)
    -> GpSimd partition_broadcast -> DVE reciprocal approx -> multiply during
    PSUM evacuation; 1/sqrt(64) folded into the exp activation scale
  - out projection contracts per-head (K=64) over host-split Wo slices; bias
    bo added after the ReduceScatter on each core's chunk
"""

import sys

for _p in ("/opt/trn_rl_repo", "/root/.axon_site/_ro/trn_rl_repo"):
    if _p not in sys.path:
        sys.path.insert(0, _p)

import numpy as np

import concourse.bass as bass
import concourse.tile as tile
from concourse import bacc
import concourse.mybir as mybir

B = 2
S = 2048
HID = 1024
HEADS_PER_CORE = 4
DH = 64  # head dim
HG = HEADS_PER_CORE * DH  # 256: hidden slice per core
N_CORES = 8
GROUP = 4  # cores per batch (reduction group)

F32 = mybir.dt.float32
F32R = mybir.dt.float32r
AF = mybir.ActivationFunctionType
ALU = mybir.AluOpType

KT = 128  # contraction tile (partitions)
QS = 512  # q strip width
NKT = S // KT  # 16 k-tiles
NQS = S // QS  # 4 q strips
NST = S // KT  # 16 s tiles


def build_nc():
    nc = bacc.Bacc(
        "TRN2", target_bir_lowering=False, debug=False, num_devices=N_CORES
    )

    # per-core inputs (already sharded/transposed by the host)
    xq = nc.dram_tensor("xq", [HID, S], F32, kind="ExternalInput").ap()
    xk = nc.dram_tensor("xk", [HID, S], F32, kind="ExternalInput").ap()
    xv = nc.dram_tensor("xv", [HID, S], F32, kind="ExternalInput").ap()
    wq = nc.dram_tensor("wq", [HID, HG], F32, kind="ExternalInput").ap()
    wk = nc.dram_tensor("wk", [HID, HG], F32, kind="ExternalInput").ap()
    wv = nc.dram_tensor("wv", [HID, HG], F32, kind="ExternalInput").ap()
    w2 = nc.dram_tensor("w2", [HEADS_PER_CORE, DH, HID], F32, kind="ExternalInput").ap()
    bqk = nc.dram_tensor("bqk", [2, 2, 128, 1], F32, kind="ExternalInput").ap()
    bvb = nc.dram_tensor("bvb", [128, HG], F32, kind="ExternalInput").ap()
    bob = nc.dram_tensor("bob", [128, HID], F32, kind="ExternalInput").ap()
    msk = nc.dram_tensor("msk", [128, 4, QS], F32, kind="ExternalInput").ap()

    out_chunk = nc.dram_tensor(
        "out_chunk", [S // GROUP, HID], F32, kind="ExternalOutput"
    ).ap()

    out_part = nc.dram_tensor("out_part", [S, HID], F32)
    rs_out = nc.dram_tensor("rs_out", [S // GROUP, HID], F32)

    groups = [[0, 1, 2, 3], [4, 5, 6, 7]]

    with tile.TileContext(nc) as tc:
        with (
            tc.tile_pool(name="wpool", bufs=1) as wpool,
            tc.tile_pool(name="qkv", bufs=1) as qkv,
        ):
            # ---- constants / weights ----
            wq_sb = []
            wk_sb = []
            wv_sb = []
            for k in range(8):
                t = wpool.tile([128, HG], F32R, tag=f"wq{k}")
                nc.sync.dma_start(t[:], wq[128 * k : 128 * k + 128].bitcast(F32R))
                wq_sb.append(t)
                t = wpool.tile([128, HG], F32R, tag=f"wk{k}")
                nc.sync.dma_start(t[:], wk[128 * k : 128 * k + 128].bitcast(F32R))
                wk_sb.append(t)
                t = wpool.tile([128, HG], F32R, tag=f"wv{k}")
                nc.sync.dma_start(t[:], wv[128 * k : 128 * k + 128].bitcast(F32R))
                wv_sb.append(t)
            w2_sb = []
            for h in range(HEADS_PER_CORE):
                t = wpool.tile([DH, HID], F32R, tag=f"w2{h}")
                nc.sync.dma_start(t[:], w2[h].bitcast(F32R))
                w2_sb.append(t)
            bq_sb = []
            bk_sb = []
            for m in range(2):
                t = wpool.tile([128, 1], F32, tag=f"bq{m}")
                nc.sync.dma_start(t[:], bqk[0, m])
                bq_sb.append(t)
                t = wpool.tile([128, 1], F32, tag=f"bk{m}")
                nc.sync.dma_start(t[:], bqk[1, m])
                bk_sb.append(t)
            bv_sb = wpool.tile([128, HG], F32, tag="bvb")
            nc.sync.dma_start(bv_sb[:], bvb[:])
            bo_sb = wpool.tile([128, HID], F32, tag="bob")
            nc.sync.dma_start(bo_sb[:], bob[:])
            # [1, 64] of ones: stationary operand of the rowsum-broadcast
            # outer-product matmul
            ones_sb = wpool.tile([1, DH], F32R, tag="ones")
            nc.scalar.activation(
                ones_sb[:], bo_sb[0:1, 0:DH], AF.Identity, scale=0.0, bias=1.0
            )
            mask_sb = wpool.tile([128, 4, QS], F32R, tag="msk")
            nc.sync.dma_start(mask_sb[:], msk.bitcast(F32R))

            # ---- persistent activations ----
            # QT/KT: [dh', s] with heads 2t, 2t+1 in partition halves of tile t
            qt_sb = [qkv.tile([128, S], F32R, tag=f"qt{m}", name=f"qt{m}") for m in range(2)]
            kt_sb = [qkv.tile([128, S], F32R, tag=f"kt{m}", name=f"kt{m}") for m in range(2)]
            # V natural [s, (head, dh+1)] with a ones column per head
            v_sb = [qkv.tile([128, HEADS_PER_CORE, DH + 1], F32R, tag=f"v{st}", name=f"v{st}")
                    for st in range(NST)]
            # normalized attention outputs OT, per (head, strip): [dh, q]
            ot_sb = [[qkv.tile([DH, QS], F32R, tag=f"ot{h}_{s4}", name=f"ot{h}_{s4}")
                      for s4 in range(NQS)] for h in range(HEADS_PER_CORE)]

            # ---- phase P: projections ----
            with tc.tile_pool(name="xs", bufs=4) as xs, \
                 tc.tile_pool(name="pj", bufs=2, space="PSUM") as pj, \
                 tc.tile_pool(name="pv", bufs=4, space="PSUM") as pv:
                for t in range(NQS):
                    sl = slice(QS * t, QS * t + QS)
                    # QT / KT strips, weight-stationary; k-tiles streamed
                    for w_sb, xdram, sbuf, b_sb, xtag in (
                        (wq_sb, xq, qt_sb, bq_sb, "xq"),
                        (wk_sb, xk, kt_sb, bk_sb, "xk"),
                    ):
                        ps0 = pj.tile([128, QS], F32, tag="pj", name="ps0")
                        ps1 = pj.tile([128, QS], F32, tag="pj", name="ps1")
                        pboth = (ps0, ps1)
                        for k in range(8):
                            xt_ = xs.tile([128, QS], F32R, tag=xtag, name="xt")
                            dma_eng = nc.sync if k % 2 == 0 else nc.scalar
                            dma_eng.dma_start(
                                xt_[:],
                                xdram[128 * k : 128 * k + 128, sl].bitcast(F32R),
                            )
                            for m in range(2):
                                nc.tensor.matmul(
                                    pboth[m][:],
                                    w_sb[k][:, 128 * m : 128 * m + 128],
                                    xt_[:],
                                    start=(k == 0),
                                    stop=(k == 7),
                                )
                        for m in range(2):
                            nc.scalar.activation(
                                sbuf[m][:, sl], pboth[m][:], AF.Identity,
                                bias=b_sb[m][:],
                            )
                    # V s-tiles (x-stationary); k-tiles streamed, 4 psum groups
                    psv = [pv.tile([128, HG], F32, tag="pv", name=f"psv{u}")
                           for u in range(4)]
                    for k in range(8):
                        xt_ = xs.tile([128, QS], F32R, tag="xv", name="xt")
                        dma_eng = nc.sync if k % 2 == 0 else nc.scalar
                        dma_eng.dma_start(
                            xt_[:], xv[128 * k : 128 * k + 128, sl].bitcast(F32R)
                        )
                        for u in range(4):
                            nc.tensor.matmul(
                                psv[u][:],
                                xt_[:, 128 * u : 128 * u + 128],
                                wv_sb[k][:],
                                start=(k == 0),
                                stop=(k == 7),
                            )
                    for u in range(4):
                        st = 4 * t + u
                        for h in range(HEADS_PER_CORE):
                            nc.vector.tensor_tensor(
                                v_sb[st][:, h, 0:DH],
                                psv[u][:, DH * h : DH * h + DH],
                                bv_sb[:, DH * h : DH * h + DH],
                                ALU.add,
                            )
                        nc.scalar.activation(
                            v_sb[st][:, :, DH],
                            bo_sb[:, 0:HEADS_PER_CORE],
                            AF.Identity,
                            scale=0.0,
                            bias=1.0,
                        )

            # ---- phases A+O: attention, out-projection, chunked RS ----
            # s4-outer so each q-strip's output rows complete early and the
            # ReduceScatter chunks overlap with later strips' compute.
            with tc.tile_pool(name="attn", bufs=8) as attn_pool, \
                 tc.tile_pool(name="norm", bufs=2) as norm_pool, \
                 tc.tile_pool(name="osb", bufs=3) as osb_pool, \
                 tc.tile_pool(name="ps", bufs=4, space="PSUM") as psp, \
                 tc.tile_pool(name="po", bufs=2, space="PSUM") as pop, \
                 tc.tile_pool(name="pu", bufs=2, space="PSUM") as pup:
                for s4 in range(NQS):
                    qsl = slice(QS * s4, QS * s4 + QS)
                    nkt = 4 * s4 + 4
                    for pair in range(2):
                        # heads 2*pair and 2*pair+1 packed into PE row groups
                        pso = [
                            pop.tile([DH + 1, QS], F32, tag="po", name=f"pso{hh}")
                            for hh in range(2)
                        ]
                        ats = [None, None]
                        for j in range(nkt):
                            for hh in range(2):
                                h = 2 * pair + hh
                                hp = 64 * hh
                                pss = psp.tile([128, QS], F32, tag="ps", name="pss")
                                nc.tensor.matmul(
                                    pss[:],
                                    kt_sb[pair][hp : hp + 64, 128 * j : 128 * j + 128],
                                    qt_sb[pair][hp : hp + 64, qsl],
                                    start=True,
                                    stop=True,
                                )
                                at = attn_pool.tile([128, QS], F32R, tag="at", name="at")
                                nc.scalar.activation(
                                    at[:], pss[:], AF.Exp, scale=1.0 / 8.0
                                )
                                ats[hh] = at
                            # the two heads' mask+attnV
                            for hh in range(2):
                                h = 2 * pair + hh
                                at = ats[hh]
                                if j >= 4 * s4:
                                    i = j - 4 * s4
                                    nc.vector.tensor_tensor(
                                        at[:], at[:], mask_sb[:, i, :],
                                        ALU.mult,
                                    )
                                nc.tensor.matmul(
                                    pso[hh][:],
                                    v_sb[j][:, h, :],
                                    at[:],
                                    start=(j == 0),
                                    stop=(j == nkt - 1),
                                )
                        for hh in range(2):
                            h = 2 * pair + hh
                            rs = norm_pool.tile([1, QS], F32R, tag="rs", name="rs")
                            nc.scalar.activation(rs[:], pso[hh][DH : DH + 1], AF.Copy)
                            # broadcast rowsums to 64 partitions via a K=1
                            # outer-product matmul (PE), slot shared with "ps"
                            rbc = psp.tile([64, QS], F32, tag="ps", name="rbc")
                            nc.tensor.matmul(
                                rbc[:], ones_sb[:], rs[:], start=True, stop=True
                            )
                            rrec = norm_pool.tile([64, QS], F32, tag="rrec", name="rrec")
                            nc.vector.reciprocal_approx_fast(rrec[:], rbc[:])
                            nc.vector.tensor_tensor(
                                ot_sb[h][s4][:], pso[hh][0:DH], rrec[:], ALU.mult
                            )

                    # out-projection for this strip's four s-tiles
                    for u in range(4):
                        st = 4 * s4 + u
                        o = u * 128
                        for eh in range(2):
                            esl = slice(QS * eh, QS * eh + QS)
                            ps = pup.tile([128, QS], F32, tag="pu", name="psu")
                            for h in range(HEADS_PER_CORE):
                                nc.tensor.matmul(
                                    ps[:],
                                    ot_sb[h][s4][:, o : o + 128],
                                    w2_sb[h][:, esl],
                                    start=(h == 0),
                                    stop=(h == HEADS_PER_CORE - 1),
                                )
                            osb = osb_pool.tile([128, QS], F32, tag="osb", name="osb")
                            nc.scalar.activation(osb[:], ps[:], AF.Copy)
                            nc.sync.dma_start(
                                out_part[128 * st : 128 * st + 128, esl], osb[:]
                            )

                    # reduce-scatter this strip's 512 output rows; core with
                    # group rank r receives rows [512*s4 + 128*r, +128)
                    nc.gpsimd.collective_compute(
                        "ReduceScatter",
                        ALU.add,
                        replica_groups=groups,
                        ins=[out_part[QS * s4 : QS * s4 + QS].opt()],
                        outs=[rs_out[128 * s4 : 128 * s4 + 128].opt()],
                    )
                    # post-RS path entirely on GpSimd so no compute engine
                    # (PE/ACT/DVE) ever waits on the collective
                    t_in = osb_pool.tile([128, HID], F32, tag="rs_in", name="rs_in")
                    nc.gpsimd.dma_start(t_in[:], rs_out[128 * s4 : 128 * s4 + 128])
                    t_out = osb_pool.tile([128, HID], F32, tag="rs_bias", name="rs_b")
                    nc.gpsimd.tensor_tensor(t_out[:], t_in[:], bo_sb[:], ALU.add)
                    nc.gpsimd.dma_start(out_chunk[128 * s4 : 128 * s4 + 128], t_out[:])

    nc.compile()
    return nc


_NC = None
_RUNNER = None


def _get_runner():
    """Build the compiled 8-core PJRT callable once and cache it."""
    global _NC, _RUNNER
    if _RUNNER is not None:
        return _RUNNER

    import jax
    import numpy as _np
    from jax.sharding import Mesh, PartitionSpec
    from jax.experimental.shard_map import shard_map
    from concourse.bass2jax import (
        _bass_exec_p,
        install_neuronx_cc_hook,
        partition_id_tensor,
    )

    _NC = build_nc()
    nc = _NC
    install_neuronx_cc_hook()

    partition_name = nc.partition_id_tensor.name if nc.partition_id_tensor else None
    in_names = []
    out_names = []
    out_avals = []
    zero_outs = []
    for alloc in nc.m.functions[0].allocations:
        if not isinstance(alloc, mybir.MemoryLocationSet):
            continue
        name = alloc.memorylocations[0].name
        if alloc.kind == "ExternalInput":
            if name != partition_name:
                in_names.append(name)
        elif alloc.kind == "ExternalOutput":
            shape = tuple(alloc.tensor_shape)
            dtype = mybir.dt.np(alloc.dtype)
            out_names.append(name)
            out_avals.append(jax.core.ShapedArray(shape, dtype))
            zero_outs.append(_np.zeros(shape, dtype))
    n_params = len(in_names)
    n_outs = len(out_avals)
    all_in_names = list(in_names) + list(out_names)
    if partition_name is not None:
        all_in_names.append(partition_name)
    donate = tuple(range(n_params, n_params + n_outs))

    def _body(*args):
        operands = list(args)
        if partition_name is not None:
            operands.append(partition_id_tensor())
        outs = _bass_exec_p.bind(
            *operands,
            out_avals=tuple(out_avals),
            in_names=tuple(all_in_names),
            out_names=tuple(out_names),
            lowering_input_output_aliases=(),
            sim_require_finite=True,
            sim_require_nnan=True,
            nc=nc,
        )
        return tuple(outs)

    devices = jax.devices()[:N_CORES]
    mesh = Mesh(np.asarray(devices), ("core",))
    in_specs = (PartitionSpec("core"),) * (n_params + n_outs)
    out_specs = (PartitionSpec("core"),) * n_outs
    sharded = jax.jit(
        shard_map(
            _body, mesh=mesh, in_specs=in_specs, out_specs=out_specs, check_rep=False
        ),
        keep_unused=True,
    )

    def run(in_maps):
        per_core = [[_np.asarray(m[name]) for name in in_names] for m in in_maps]
        concat_in = [
            _np.concatenate([per_core[c][i] for c in range(N_CORES)], axis=0)
            for i in range(n_params)
        ]
        concat_zeros = [
            _np.zeros((N_CORES * z.shape[0], *z.shape[1:]), z.dtype)
            for z in zero_outs
        ]
        out_arrs = sharded(*concat_in, *concat_zeros)
        return [
            {
                name: _np.asarray(out_arrs[i]).reshape(
                    N_CORES, *out_avals[i].shape
                )[c]
                for i, name in enumerate(out_names)
            }
            for c in range(N_CORES)
        ]

    _RUNNER = run
    return run


def make_in_maps(query, key, value, Wq, bq, Wk, bk, Wv, bv, Wo, bo):
    query = np.asarray(query, dtype=np.float32)
    key = np.asarray(key, dtype=np.float32)
    value = np.asarray(value, dtype=np.float32)
    Wq = np.asarray(Wq, dtype=np.float32)
    bq = np.asarray(bq, dtype=np.float32)
    Wk = np.asarray(Wk, dtype=np.float32)
    bk = np.asarray(bk, dtype=np.float32)
    Wv = np.asarray(Wv, dtype=np.float32)
    bv = np.asarray(bv, dtype=np.float32)
    Wo = np.asarray(Wo, dtype=np.float32)
    bo = np.asarray(bo, dtype=np.float32)

    xqT = [np.ascontiguousarray(query[b].T) for b in range(B)]
    xkT = [np.ascontiguousarray(key[b].T) for b in range(B)]
    xvT = [np.ascontiguousarray(value[b].T) for b in range(B)]

    # diagonal-block causal masks: mask[k, i, q] = 1 if q >= k + 128*i
    k_idx = np.arange(128)[:, None, None]
    i_idx = np.arange(4)[None, :, None]
    q_idx = np.arange(QS)[None, None, :]
    masks = (q_idx >= k_idx + 128 * i_idx).astype(np.float32)

    bo_b = np.ascontiguousarray(np.broadcast_to(bo, (128, HID)))

    in_maps = []
    for c in range(N_CORES):
        b = c // GROUP
        g = c % GROUP
        hsl = slice(HG * g, HG * g + HG)
        wq_g = np.ascontiguousarray(Wq[hsl].T)  # [1024, 256]
        wk_g = np.ascontiguousarray(Wk[hsl].T)
        wv_g = np.ascontiguousarray(Wv[hsl].T)
        # w2[h] = Wo[:, g*256 + 64h : +64].T  -> [64, 1024]
        w2_g = np.ascontiguousarray(
            Wo[:, hsl].T.reshape(HEADS_PER_CORE, DH, HID)
        )
        bqk_g = np.stack(
            [bq[hsl].reshape(2, 128), bk[hsl].reshape(2, 128)]
        )  # [2, 2, 128]
        bv_b = np.ascontiguousarray(np.broadcast_to(bv[hsl], (128, HG)))
        in_maps.append(
            {
                "xq": xqT[b],
                "xk": xkT[b],
                "xv": xvT[b],
                "wq": wq_g,
                "wk": wk_g,
                "wv": wv_g,
                "w2": w2_g,
                "bqk": bqk_g,
                "bvb": bv_b,
                "bob": bo_b,
                "msk": masks,
            }
        )
    return in_maps


def assemble_output(results):
    # core with group rank r holds, for each strip j, global rows
    # 512*j + 128*r .. +128 in its out_chunk block j
    out = np.empty((B, S, HID), dtype=np.float32)
    for b in range(B):
        for r in range(GROUP):
            chunk = results[GROUP * b + r]["out_chunk"]
            for j in range(NQS):
                out[b, QS * j + 128 * r : QS * j + 128 * r + 128] = chunk[
                    128 * j : 128 * j + 128
                ]
    return out


def kernel(**inputs) -> np.ndarray:
    in_maps = make_in_maps(**inputs)
    run = _get_runner()
    results = run(in_maps)
    return assemble_output(results)


if __name__ == "__main__":
    import reference

    inputs = {k: np.asarray(v) for k, v in reference.setup_inputs().items()}
    got = kernel(**inputs)
    want = np.asarray(reference.reference(**inputs))
    err = np.linalg.norm(got - want) / np.linalg.norm(want)
    print("Relative error:", err)


# revision 29
# speedup vs baseline: 1.1442x; 1.0514x over previous
"""Multi-head causal attention (B=2, S=2048, H=1024, 16 heads) on 8 TRN2
NeuronCores.

Sharding: core c in 0..7 handles batch b = c // 4 and head group g = c % 4
(heads 4g..4g+3).  Each core computes Q/K/V projections for its 4 heads,
causal attention, and the partial output projection through its column slice
of Wo.  The 4 cores of a batch ReduceScatter(add) their [2048, 1024] partials
so core i of the group ends up with rows 512*i..512*i+512 fully reduced; the
host concatenates the chunks.

Device dataflow (per core, all matmuls in float32r):
  - activations pre-transposed on host to [1024, 2048] (feature-major) since
    the PE contracts over the partition dim
  - QT/KT [256, 2048] via weight-stationary matmuls, bias fused in the ACT
    PSUM->SBUF evacuation; V [2048, 256] natural with a fused ones column per
    head (rowsums fall out of the attention-value matmul)
  - scores computed transposed (scoresT[k, q]) so the exp'd tiles feed the
    attention-value matmul directly as the stationary operand, no transposes
  - causal handled by skipping fully-masked 128x512 blocks and multiplying
    the 4 diagonal-block patterns with precomputed 0/1 masks
  - softmax normalization: rowsum row (partition 64 of the attnV PSUM tile)
    -> GpSimd partition_broadcast -> DVE reciprocal approx -> multiply during
    PSUM evacuation; 1/sqrt(64) folded into the exp activation scale
  - out projection contracts per-head (K=64) over host-split Wo slices; bias
    bo added after the ReduceScatter on each core's chunk
"""

import sys

for _p in ("/opt/trn_rl_repo", "/root/.axon_site/_ro/trn_rl_repo"):
    if _p not in sys.path:
        sys.path.insert(0, _p)

import numpy as np

import concourse.bass as bass
import concourse.tile as tile
from concourse import bacc
import concourse.mybir as mybir

B = 2
S = 2048
HID = 1024
HEADS_PER_CORE = 4
DH = 64  # head dim
HG = HEADS_PER_CORE * DH  # 256: hidden slice per core
N_CORES = 8
GROUP = 4  # cores per batch (reduction group)

F32 = mybir.dt.float32
F32R = mybir.dt.float32r
AF = mybir.ActivationFunctionType
ALU = mybir.AluOpType

KT = 128  # contraction tile (partitions)
QS = 512  # q strip width
NKT = S // KT  # 16 k-tiles
NQS = S // QS  # 4 q strips
NST = S // KT  # 16 s tiles


def build_nc():
    nc = bacc.Bacc(
        "TRN2", target_bir_lowering=False, debug=False, num_devices=N_CORES
    )

    # per-core inputs (already sharded/transposed by the host)
    xq = nc.dram_tensor("xq", [HID, S], F32, kind="ExternalInput").ap()
    xk = nc.dram_tensor("xk", [HID, S], F32, kind="ExternalInput").ap()
    xv = nc.dram_tensor("xv", [HID, S], F32, kind="ExternalInput").ap()
    wq = nc.dram_tensor("wq", [HID, HG], F32, kind="ExternalInput").ap()
    wk = nc.dram_tensor("wk", [HID, HG], F32, kind="ExternalInput").ap()
    wv = nc.dram_tensor("wv", [HID, HG], F32, kind="ExternalInput").ap()
    w2 = nc.dram_tensor("w2", [HEADS_PER_CORE, DH, HID], F32, kind="ExternalInput").ap()
    bqk = nc.dram_tensor("bqk", [2, 2, 128, 1], F32, kind="ExternalInput").ap()
    bvb = nc.dram_tensor("bvb", [128, HG], F32, kind="ExternalInput").ap()
    bob = nc.dram_tensor("bob", [128, HID], F32, kind="ExternalInput").ap()
    msk = nc.dram_tensor("msk", [128, 4, QS], F32, kind="ExternalInput").ap()

    out_chunk = nc.dram_tensor(
        "out_chunk", [S // GROUP, HID], F32, kind="ExternalOutput"
    ).ap()

    out_part = nc.dram_tensor("out_part", [S, HID], F32)
    rs_out = nc.dram_tensor("rs_out", [S // GROUP, HID], F32)
    cc_warm_in = nc.dram_tensor("cc_warm_in", [4, 128], F32)
    cc_warm_out = nc.dram_tensor("cc_warm_out", [1, 128], F32)

    groups = [[0, 1, 2, 3], [4, 5, 6, 7]]

    with tile.TileContext(nc) as tc:
        with (
            tc.tile_pool(name="wpool", bufs=1) as wpool,
            tc.tile_pool(name="qkv", bufs=1) as qkv,
        ):
            # ---- constants / weights ----
            wq_sb = []
            wk_sb = []
            wv_sb = []
            for k in range(8):
                t = wpool.tile([128, HG], F32R, tag=f"wq{k}")
                nc.sync.dma_start(t[:], wq[128 * k : 128 * k + 128].bitcast(F32R))
                wq_sb.append(t)
                t = wpool.tile([128, HG], F32R, tag=f"wk{k}")
                nc.sync.dma_start(t[:], wk[128 * k : 128 * k + 128].bitcast(F32R))
                wk_sb.append(t)
                t = wpool.tile([128, HG], F32R, tag=f"wv{k}")
                nc.sync.dma_start(t[:], wv[128 * k : 128 * k + 128].bitcast(F32R))
                wv_sb.append(t)
            w2_sb = []
            for h in range(HEADS_PER_CORE):
                t = wpool.tile([DH, HID], F32R, tag=f"w2{h}")
                nc.sync.dma_start(t[:], w2[h].bitcast(F32R))
                w2_sb.append(t)
            bq_sb = []
            bk_sb = []
            for m in range(2):
                t = wpool.tile([128, 1], F32, tag=f"bq{m}")
                nc.sync.dma_start(t[:], bqk[0, m])
                bq_sb.append(t)
                t = wpool.tile([128, 1], F32, tag=f"bk{m}")
                nc.sync.dma_start(t[:], bqk[1, m])
                bk_sb.append(t)
            bv_sb = wpool.tile([128, HG], F32, tag="bvb")
            nc.sync.dma_start(bv_sb[:], bvb[:])
            bo_sb = wpool.tile([128, HID], F32, tag="bob")
            nc.sync.dma_start(bo_sb[:], bob[:])
            # [1, 64] of ones: stationary operand of the rowsum-broadcast
            # outer-product matmul
            ones_sb = wpool.tile([1, DH], F32R, tag="ones")
            nc.scalar.activation(
                ones_sb[:], bo_sb[0:1, 0:DH], AF.Identity, scale=0.0, bias=1.0
            )
            # tiny dummy collective: warms the CC stream so the first real
            # ReduceScatter doesn't pay stream-startup costs
            zt = wpool.tile([4, 128], F32, tag="zt")
            nc.gpsimd.memset(zt[:], 0.0)
            nc.gpsimd.dma_start(cc_warm_in[:], zt[:])
            nc.gpsimd.collective_compute(
                "ReduceScatter",
                ALU.add,
                replica_groups=groups,
                ins=[cc_warm_in[:]],
                outs=[cc_warm_out[:]],
            )
            mask_sb = wpool.tile([128, 4, QS], F32R, tag="msk")
            nc.sync.dma_start(mask_sb[:], msk.bitcast(F32R))

            # ---- persistent activations ----
            # QT/KT: [dh', s] with heads 2t, 2t+1 in partition halves of tile t
            qt_sb = [qkv.tile([128, S], F32R, tag=f"qt{m}", name=f"qt{m}") for m in range(2)]
            kt_sb = [qkv.tile([128, S], F32R, tag=f"kt{m}", name=f"kt{m}") for m in range(2)]
            # V natural [s, (head, dh+1)] with a ones column per head
            v_sb = [qkv.tile([128, HEADS_PER_CORE, DH + 1], F32R, tag=f"v{st}", name=f"v{st}")
                    for st in range(NST)]
            # normalized attention outputs OT, per (head, strip): [dh, q]
            ot_sb = [[qkv.tile([DH, QS], F32R, tag=f"ot{h}_{s4}", name=f"ot{h}_{s4}")
                      for s4 in range(NQS)] for h in range(HEADS_PER_CORE)]

            # ---- phase P: projections ----
            with tc.tile_pool(name="xs", bufs=4) as xs, \
                 tc.tile_pool(name="pj", bufs=2, space="PSUM") as pj, \
                 tc.tile_pool(name="pv", bufs=4, space="PSUM") as pv:
                for t in range(NQS):
                    sl = slice(QS * t, QS * t + QS)
                    # QT / KT strips, weight-stationary; k-tiles streamed
                    for w_sb, xdram, sbuf, b_sb, xtag in (
                        (wq_sb, xq, qt_sb, bq_sb, "xq"),
                        (wk_sb, xk, kt_sb, bk_sb, "xk"),
                    ):
                        ps0 = pj.tile([128, QS], F32, tag="pj", name="ps0")
                        ps1 = pj.tile([128, QS], F32, tag="pj", name="ps1")
                        pboth = (ps0, ps1)
                        for k in range(8):
                            xt_ = xs.tile([128, QS], F32R, tag=xtag, name="xt")
                            dma_eng = nc.sync if k % 2 == 0 else nc.scalar
                            dma_eng.dma_start(
                                xt_[:],
                                xdram[128 * k : 128 * k + 128, sl].bitcast(F32R),
                            )
                            for m in range(2):
                                nc.tensor.matmul(
                                    pboth[m][:],
                                    w_sb[k][:, 128 * m : 128 * m + 128],
                                    xt_[:],
                                    start=(k == 0),
                                    stop=(k == 7),
                                )
                        for m in range(2):
                            nc.scalar.activation(
                                sbuf[m][:, sl], pboth[m][:], AF.Identity,
                                bias=b_sb[m][:],
                            )
                    # V s-tiles (x-stationary); k-tiles streamed, 4 psum groups
                    psv = [pv.tile([128, HG], F32, tag="pv", name=f"psv{u}")
                           for u in range(4)]
                    for k in range(8):
                        xt_ = xs.tile([128, QS], F32R, tag="xv", name="xt")
                        dma_eng = nc.sync if k % 2 == 0 else nc.scalar
                        dma_eng.dma_start(
                            xt_[:], xv[128 * k : 128 * k + 128, sl].bitcast(F32R)
                        )
                        for u in range(4):
                            nc.tensor.matmul(
                                psv[u][:],
                                xt_[:, 128 * u : 128 * u + 128],
                                wv_sb[k][:],
                                start=(k == 0),
                                stop=(k == 7),
                            )
                    for u in range(4):
                        st = 4 * t + u
                        for h in range(HEADS_PER_CORE):
                            nc.vector.tensor_tensor(
                                v_sb[st][:, h, 0:DH],
                                psv[u][:, DH * h : DH * h + DH],
                                bv_sb[:, DH * h : DH * h + DH],
                                ALU.add,
                            )
                        nc.scalar.activation(
                            v_sb[st][:, :, DH],
                            bo_sb[:, 0:HEADS_PER_CORE],
                            AF.Identity,
                            scale=0.0,
                            bias=1.0,
                        )

            # ---- phases A+O: attention, out-projection, chunked RS ----
            # s4-outer so each q-strip's output rows complete early and the
            # ReduceScatter chunks overlap with later strips' compute.
            with tc.tile_pool(name="attn", bufs=8) as attn_pool, \
                 tc.tile_pool(name="norm", bufs=2) as norm_pool, \
                 tc.tile_pool(name="osb", bufs=3) as osb_pool, \
                 tc.tile_pool(name="ps", bufs=4, space="PSUM") as psp, \
                 tc.tile_pool(name="po", bufs=2, space="PSUM") as pop, \
                 tc.tile_pool(name="pu", bufs=2, space="PSUM") as pup:
                for s4 in range(NQS):
                    qsl = slice(QS * s4, QS * s4 + QS)
                    nkt = 4 * s4 + 4
                    for pair in range(2):
                        # heads 2*pair and 2*pair+1 packed into PE row groups
                        pso = [
                            pop.tile([DH + 1, QS], F32, tag="po", name=f"pso{hh}")
                            for hh in range(2)
                        ]
                        ats = {}

                        def do_scores(j):
                            ats[j] = []
                            for hh in range(2):
                                hp = 64 * hh
                                pss = psp.tile([128, QS], F32, tag="ps", name="pss")
                                nc.tensor.matmul(
                                    pss[:],
                                    kt_sb[pair][hp : hp + 64, 128 * j : 128 * j + 128],
                                    qt_sb[pair][hp : hp + 64, qsl],
                                    start=True,
                                    stop=True,
                                )
                                at = attn_pool.tile(
                                    [128, QS], F32R, tag="at", name="at"
                                )
                                nc.scalar.activation(
                                    at[:], pss[:], AF.Exp, scale=1.0 / 8.0
                                )
                                if j >= 4 * s4:
                                    i = j - 4 * s4
                                    nc.vector.tensor_tensor(
                                        at[:], at[:], mask_sb[:, i, :], ALU.mult
                                    )
                                ats[j].append(at)

                        # scores pipelined one k-tile ahead of attnV so the
                        # in-order PE always has score matmuls queued while
                        # ACT produces the exp tiles
                        do_scores(0)
                        for j in range(nkt):
                            if j + 1 < nkt:
                                do_scores(j + 1)
                            for hh in range(2):
                                h = 2 * pair + hh
                                nc.tensor.matmul(
                                    pso[hh][:],
                                    v_sb[j][:, h, :],
                                    ats[j][hh][:],
                                    start=(j == 0),
                                    stop=(j == nkt - 1),
                                )
                            del ats[j]
                        for hh in range(2):
                            h = 2 * pair + hh
                            rs = norm_pool.tile([1, QS], F32R, tag="rs", name="rs")
                            nc.scalar.activation(rs[:], pso[hh][DH : DH + 1], AF.Copy)
                            # broadcast rowsums to 64 partitions via a K=1
                            # outer-product matmul (PE), slot shared with "ps"
                            rbc = psp.tile([64, QS], F32, tag="ps", name="rbc")
                            nc.tensor.matmul(
                                rbc[:], ones_sb[:], rs[:], start=True, stop=True
                            )
                            rrec = norm_pool.tile([64, QS], F32, tag="rrec", name="rrec")
                            nc.vector.reciprocal_approx_fast(rrec[:], rbc[:])
                            nc.vector.tensor_tensor(
                                ot_sb[h][s4][:], pso[hh][0:DH], rrec[:], ALU.mult
                            )

                    # out-projection for this strip's four s-tiles
                    for u in range(4):
                        st = 4 * s4 + u
                        o = u * 128
                        for eh in range(2):
                            esl = slice(QS * eh, QS * eh + QS)
                            ps = pup.tile([128, QS], F32, tag="pu", name="psu")
                            for h in range(HEADS_PER_CORE):
                                nc.tensor.matmul(
                                    ps[:],
                                    ot_sb[h][s4][:, o : o + 128],
                                    w2_sb[h][:, esl],
                                    start=(h == 0),
                                    stop=(h == HEADS_PER_CORE - 1),
                                )
                            osb = osb_pool.tile([128, QS], F32, tag="osb", name="osb")
                            nc.vector.tensor_copy(osb[:], ps[:])
                            nc.sync.dma_start(
                                out_part[128 * st : 128 * st + 128, esl], osb[:]
                            )
                        # reduce-scatter finished output rows; the last strip
                        # is split in half so the final collective is smaller.
                        # Core with group rank r receives the chunk's r-th
                        # quarter; its out_chunk row offset is r0 // 4.
                        if s4 < NQS - 1:
                            chunks = [(QS * s4, QS)] if u == 3 else []
                        else:
                            chunks = (
                                [(QS * s4, QS // 2)] if u == 1
                                else [(QS * s4 + QS // 2, QS // 2)] if u == 3
                                else []
                            )
                        for r0, rn in chunks:
                            q = rn // 4
                            nc.gpsimd.collective_compute(
                                "ReduceScatter",
                                ALU.add,
                                replica_groups=groups,
                                ins=[out_part[r0 : r0 + rn].opt()],
                                outs=[rs_out[r0 // 4 : r0 // 4 + q].opt()],
                            )
                            # post-RS path entirely on GpSimd so no compute
                            # engine (PE/ACT/DVE) ever waits on the collective
                            t_in = osb_pool.tile(
                                [128, HID], F32, tag="rs_in", name="rs_in"
                            )
                            nc.gpsimd.dma_start(
                                t_in[0:q], rs_out[r0 // 4 : r0 // 4 + q]
                            )
                            t_out = osb_pool.tile(
                                [128, HID], F32, tag="rs_bias", name="rs_b"
                            )
                            nc.gpsimd.tensor_tensor(
                                t_out[0:q], t_in[0:q], bo_sb[0:q], ALU.add
                            )
                            nc.gpsimd.dma_start(
                                out_chunk[r0 // 4 : r0 // 4 + q], t_out[0:q]
                            )

    nc.compile()
    return nc


_NC = None
_RUNNER = None


def _get_runner():
    """Build the compiled 8-core PJRT callable once and cache it."""
    global _NC, _RUNNER
    if _RUNNER is not None:
        return _RUNNER

    import jax
    import numpy as _np
    from jax.sharding import Mesh, PartitionSpec
    from jax.experimental.shard_map import shard_map
    from concourse.bass2jax import (
        _bass_exec_p,
        install_neuronx_cc_hook,
        partition_id_tensor,
    )

    _NC = build_nc()
    nc = _NC
    install_neuronx_cc_hook()

    partition_name = nc.partition_id_tensor.name if nc.partition_id_tensor else None
    in_names = []
    out_names = []
    out_avals = []
    zero_outs = []
    for alloc in nc.m.functions[0].allocations:
        if not isinstance(alloc, mybir.MemoryLocationSet):
            continue
        name = alloc.memorylocations[0].name
        if alloc.kind == "ExternalInput":
            if name != partition_name:
                in_names.append(name)
        elif alloc.kind == "ExternalOutput":
            shape = tuple(alloc.tensor_shape)
            dtype = mybir.dt.np(alloc.dtype)
            out_names.append(name)
            out_avals.append(jax.core.ShapedArray(shape, dtype))
            zero_outs.append(_np.zeros(shape, dtype))
    n_params = len(in_names)
    n_outs = len(out_avals)
    all_in_names = list(in_names) + list(out_names)
    if partition_name is not None:
        all_in_names.append(partition_name)
    donate = tuple(range(n_params, n_params + n_outs))

    def _body(*args):
        operands = list(args)
        if partition_name is not None:
            operands.append(partition_id_tensor())
        outs = _bass_exec_p.bind(
            *operands,
            out_avals=tuple(out_avals),
            in_names=tuple(all_in_names),
            out_names=tuple(out_names),
            lowering_input_output_aliases=(),
            sim_require_finite=True,
            sim_require_nnan=True,
            nc=nc,
        )
        return tuple(outs)

    devices = jax.devices()[:N_CORES]
    mesh = Mesh(np.asarray(devices), ("core",))
    in_specs = (PartitionSpec("core"),) * (n_params + n_outs)
    out_specs = (PartitionSpec("core"),) * n_outs
    sharded = jax.jit(
        shard_map(
            _body, mesh=mesh, in_specs=in_specs, out_specs=out_specs, check_rep=False
        ),
        keep_unused=True,
    )

    def run(in_maps):
        per_core = [[_np.asarray(m[name]) for name in in_names] for m in in_maps]
        concat_in = [
            _np.concatenate([per_core[c][i] for c in range(N_CORES)], axis=0)
            for i in range(n_params)
        ]
        concat_zeros = [
            _np.zeros((N_CORES * z.shape[0], *z.shape[1:]), z.dtype)
            for z in zero_outs
        ]
        out_arrs = sharded(*concat_in, *concat_zeros)
        return [
            {
                name: _np.asarray(out_arrs[i]).reshape(
                    N_CORES, *out_avals[i].shape
                )[c]
                for i, name in enumerate(out_names)
            }
            for c in range(N_CORES)
        ]

    _RUNNER = run
    return run


def make_in_maps(query, key, value, Wq, bq, Wk, bk, Wv, bv, Wo, bo):
    query = np.asarray(query, dtype=np.float32)
    key = np.asarray(key, dtype=np.float32)
    value = np.asarray(value, dtype=np.float32)
    Wq = np.asarray(Wq, dtype=np.float32)
    bq = np.asarray(bq, dtype=np.float32)
    Wk = np.asarray(Wk, dtype=np.float32)
    bk = np.asarray(bk, dtype=np.float32)
    Wv = np.asarray(Wv, dtype=np.float32)
    bv = np.asarray(bv, dtype=np.float32)
    Wo = np.asarray(Wo, dtype=np.float32)
    bo = np.asarray(bo, dtype=np.float32)

    xqT = [np.ascontiguousarray(query[b].T) for b in range(B)]
    xkT = [np.ascontiguousarray(key[b].T) for b in range(B)]
    xvT = [np.ascontiguousarray(value[b].T) for b in range(B)]

    # diagonal-block causal masks: mask[k, i, q] = 1 if q >= k + 128*i
    k_idx = np.arange(128)[:, None, None]
    i_idx = np.arange(4)[None, :, None]
    q_idx = np.arange(QS)[None, None, :]
    masks = (q_idx >= k_idx + 128 * i_idx).astype(np.float32)

    bo_b = np.ascontiguousarray(np.broadcast_to(bo, (128, HID)))

    in_maps = []
    for c in range(N_CORES):
        b = c // GROUP
        g = c % GROUP
        hsl = slice(HG * g, HG * g + HG)
        wq_g = np.ascontiguousarray(Wq[hsl].T)  # [1024, 256]
        wk_g = np.ascontiguousarray(Wk[hsl].T)
        wv_g = np.ascontiguousarray(Wv[hsl].T)
        # w2[h] = Wo[:, g*256 + 64h : +64].T  -> [64, 1024]
        w2_g = np.ascontiguousarray(
            Wo[:, hsl].T.reshape(HEADS_PER_CORE, DH, HID)
        )
        bqk_g = np.stack(
            [bq[hsl].reshape(2, 128), bk[hsl].reshape(2, 128)]
        )  # [2, 2, 128]
        bv_b = np.ascontiguousarray(np.broadcast_to(bv[hsl], (128, HG)))
        in_maps.append(
            {
                "xq": xqT[b],
                "xk": xkT[b],
                "xv": xvT[b],
                "wq": wq_g,
                "wk": wk_g,
                "wv": wv_g,
                "w2": w2_g,
                "bqk": bqk_g,
                "bvb": bv_b,
                "bob": bo_b,
                "msk": masks,
            }
        )
    return in_maps


RS_CHUNKS = [(0, 512), (512, 512), (1024, 512), (1536, 256), (1792, 256)]


def assemble_output(results):
    # for RS chunk (r0, rn), core with group rank r holds global rows
    # [r0 + (rn//4)*r, +rn//4) at out_chunk rows [r0//4, +rn//4)
    out = np.empty((B, S, HID), dtype=np.float32)
    for b in range(B):
        for r in range(GROUP):
            chunk = results[GROUP * b + r]["out_chunk"]
            for r0, rn in RS_CHUNKS:
                q = rn // 4
                out[b, r0 + q * r : r0 + q * (r + 1)] = chunk[
                    r0 // 4 : r0 // 4 + q
                ]
    return out


def kernel(**inputs) -> np.ndarray:
    in_maps = make_in_maps(**inputs)
    run = _get_runner()
    results = run(in_maps)
    return assemble_output(results)


if __name__ == "__main__":
    import reference

    inputs = {k: np.asarray(v) for k, v in reference.setup_inputs().items()}
    got = kernel(**inputs)
    want = np.asarray(reference.reference(**inputs))
    err = np.linalg.norm(got - want) / np.linalg.norm(want)
    print("Relative error:", err)
